# revision 1
# baseline (speedup 1.0000x reference)
"""Trainium2 Bass kernel for nn_Attention_84851373900515 (gnn message passing).

Reference computation (per (b, t) slice, R=2048 regions, D=64, K=16 neighbors):
    q = data @ wq                       # (R, D)
    k = data[neigh] @ wk = (data @ wk)[neigh]   # project-then-gather
    scores[r, j] = q[r] . k[neigh[r, j]]
    attn = softmax_j(scores)
    ctx[r] = sum_j attn[r, j] * k[neigh[r, j]]
    out = sigmoid((q + ctx) @ wd_s)

Sharding: data-parallel over the 48 (b, t) slices, 6 per core, processed as
3 groups of 2 slices packed side by side ("b2" packing) so gathered rows are
256 bytes (dma_gather minimum).

Pipeline per core:
  A. PE computes q/k projections from host-pretransposed data (bf16),
     k-projections are written back to a per-core HBM scratch.
  B. Per (group, region-tile): dma_gather pulls the 2048 neighbor rows
     (16 per region) from HBM into SBUF; DVE computes scores (mult+reduce),
     ACT does exp (softmax), DVE/ACT compute the attention-weighted context,
     PE transposes and applies the final wd_s projection, ACT applies
     sigmoid, PE transposes back, and the result is DMA'd out fp32.
"""

import sys

sys.path.insert(0, "/opt/trn_rl_repo")

import numpy as np

LAST_RESULTS = None  # BassKernelResults of the most recent kernel() call

B, T, R, D, K = 4, 12, 2048, 64, 16
NBT = B * T          # 48 (b, t) slices
NCORES = 8
SPC = NBT // NCORES  # 6 slices per core
NG = SPC // 2        # 3 groups of 2 slices
NT = R // 128        # 16 region tiles
P = 128


def _patch_tile_compat():
    """The walrus bundled with the installed neuronxcc (which the axon
    bass2jax path compiles through) cannot encode (a) the raw-ISA
    EVENT_SEMAPHORE_RANGE_CLEAR instruction and (b) control instructions
    carrying more than one semaphore wait. Patch Tile's kernel tail:
    skip the semaphore/DMA hardware reset (each compiled NEFF here runs
    exactly once) and split the tail drain's accumulated waits into
    single-wait EventSemaphore instructions."""
    import concourse.bass as bass
    import concourse.mybir as mybir
    import concourse.tile as tile
    from concourse.vector_clock import ScopedClock

    if getattr(tile.TileContext, "_ant_compat_patched", False):
        return

    def clear_and_free(self, sems):
        if not sems:
            return
        sem_nums = [s.num if hasattr(s, "num") else s for s in sems]
        self._state.prepend_free_semaphores(sem_nums)
        for poison_set in self._tile_sem_poison_stack:
            poison_set.update(sem_nums)

    bass.Bass.clear_and_free_semaphores = clear_and_free

    def drain_and_barrier(self, tick_clock, wait_clock):
        nc = self.nc
        drain_inst = nc.sync.drain()
        wait_clock.add_sem_waits(
            drain_inst.ins, ScopedClock({None: tick_clock.global_clock})
        )
        mi = drain_inst.ins
        si = mi.sync_info
        if si is not None and len(si.on_wait) > 1:
            waits = list(si.on_wait)
            mi.sync_info = mybir.SyncInfo(
                on_wait=[], on_update=list(si.on_update)
            )
            for w in waits:
                ev = mybir.InstEventSemaphore(
                    name=nc.get_next_instruction_name(),
                    engine=mybir.EngineType.SP,
                    ins=[],
                    outs=[],
                    sync_info=mybir.SyncInfo(on_wait=[w], on_update=[]),
                )
                self._add_instruction(ev)
        nc.all_engine_barrier()
        assert self.sems is not None
        popped = nc._tile_sem_poison_stack.pop()
        assert popped is self._sem_poison
        nc.clear_and_free_semaphores(list(self.sems.allocated().values()))
        nc.all_engine_barrier()

    tile.TileContext._drain_and_barrier = drain_and_barrier
    tile.TileContext._ant_compat_patched = True


def _hoist_multiwaits(nc):
    """Split semaphore waits that exceed what the installed walrus can
    encode per instruction into standalone single-wait EventSemaphore
    instructions on the same engine, inserted immediately before."""
    import concourse.mybir as mybir

    for f in nc.m.functions:
        for blk in f.blocks:
            out = []
            changed = False
            for inst in blk.instructions:
                si = inst.sync_info
                limit = 1
                if si is not None and len(si.on_wait) > limit:
                    waits = list(si.on_wait)
                    keep, hoist = waits[:limit], waits[limit:]
                    for w in hoist:
                        ev = mybir.InstEventSemaphore(
                            name=nc.get_next_instruction_name(),
                            engine=inst.engine,
                            ins=[],
                            outs=[],
                            sync_info=mybir.SyncInfo(on_wait=[w], on_update=[]),
                        )
                        out.append(ev)
                    inst.sync_info = mybir.SyncInfo(
                        on_wait=keep, on_update=list(si.on_update)
                    )
                    changed = True
                out.append(inst)
            if changed:
                blk.instructions = out


def _build_bass():
    from contextlib import ExitStack

    import concourse.bass as bass
    import concourse.mybir as mybir
    import concourse.tile as tile
    from concourse.masks import make_identity

    _patch_tile_compat()

    f32 = mybir.dt.float32
    bf16 = mybir.dt.bfloat16
    i32 = mybir.dt.int32
    AF = mybir.ActivationFunctionType
    OP = mybir.AluOpType
    AX = mybir.AxisListType

    nc = bass.Bass()

    # Inputs (per core)
    dat2T = [
        nc.declare_dram_parameter(f"dat2T_g{g}", [P, R], f32, isOutput=False)
        for g in range(NG)
    ]
    gidx = nc.declare_dram_parameter("gidx", [NT, P, K], i32, isOutput=False)
    wqk = nc.declare_dram_parameter("wqk", [D, 2 * D], f32, isOutput=False)
    wds = nc.declare_dram_parameter("wds", [D, D], f32, isOutput=False)
    outs = [
        nc.declare_dram_parameter(f"out_g{g}", [R, P], f32, isOutput=True)
        for g in range(NG)
    ]
    # HBM scratch holding the bf16 k-projections, gather source. Row r holds
    # all six slices' k vectors: (r, (s6, e)) = 768 B, so one 128-offset
    # indirect DMA gathers a neighbor slot for every slice at once.
    kph = nc.dram_tensor("kproj6", [R, SPC * D], bf16)

    with ExitStack() as ctx:
        tc = ctx.enter_context(tile.TileContext(nc))
        cpool = ctx.enter_context(tc.tile_pool(name="consts", bufs=1))
        apool = ctx.enter_context(tc.tile_pool(name="phaseA", bufs=2))
        qkpool = ctx.enter_context(tc.tile_pool(name="qk", bufs=1))
        gpool = ctx.enter_context(tc.tile_pool(name="gather", bufs=3))
        wpool = ctx.enter_context(tc.tile_pool(name="work", bufs=3))
        spool = ctx.enter_context(tc.tile_pool(name="small", bufs=4))
        opool = ctx.enter_context(tc.tile_pool(name="outst", bufs=1))
        pspool = ctx.enter_context(tc.tile_pool(name="ps", bufs=2, space="PSUM"))

        # ---- constants ----
        ident_bf = cpool.tile([P, P], bf16)
        make_identity(nc, ident_bf[:])
        ident_f32 = cpool.tile([P, P], f32)
        make_identity(nc, ident_f32[:])

        # Block-diagonal weights so both packed slices project in one matmul
        # (contract over all 128 partitions = (b2, e)).
        wqk_f = cpool.tile([D, 2 * D], f32)
        nc.sync.dma_start(out=wqk_f[:], in_=wqk[:])
        w2 = cpool.tile([P, 4 * D], bf16)
        nc.vector.memset(w2[:], 0.0)
        nc.vector.tensor_copy(out=w2[0:D, 0 : 2 * D], in_=wqk_f[:])
        nc.vector.tensor_copy(out=w2[D:P, 2 * D : 4 * D], in_=wqk_f[:])
        wds_f = cpool.tile([D, D], f32)
        nc.sync.dma_start(out=wds_f[:], in_=wds[:])
        wds2 = cpool.tile([P, P], bf16)
        nc.vector.memset(wds2[:], 0.0)
        nc.vector.tensor_copy(out=wds2[0:D, 0:D], in_=wds_f[:])
        nc.vector.tensor_copy(out=wds2[D:P, D:P], in_=wds_f[:])

        gidx_sb = cpool.tile([P, NT, K], i32)
        nc.sync.dma_start(
            out=gidx_sb[:], in_=gidx[:].rearrange("t rp j -> rp t j")
        )

        # qk2[r', t, b, (qk, e)] bf16 for the current group
        qk2 = [
            qkpool.tile([P, NT, 2, P], bf16, name=f"qk2_{g}", tag=f"qk2_{g}")
            for g in range(NG)
        ]

        # ---- Phase A: projections for all groups ----
        for g in range(NG):
            d2t = apool.tile([P, R], bf16, tag="d2t")
            # SWDGE dma casts fp32 -> bf16 on the way in
            nc.gpsimd.dma_start(out=d2t[:], in_=dat2T[g][:])
            for t in range(NT):
                pp = pspool.tile([P, 2, P], f32, tag="proj")
                # out[r', (b, qk, e)] = sum_(b', e') d2t[(b', e'), r'] *
                #                       w2[(b', e'), (b, qk, e)]
                nc.tensor.matmul(
                    pp[:],
                    d2t[:, 128 * t : 128 * (t + 1)],
                    w2[:],
                    start=True,
                    stop=True,
                )
                if t % 2 == 0:
                    nc.vector.tensor_copy(out=qk2[g][:, t, :, :], in_=pp[:])
                else:
                    nc.scalar.copy(out=qk2[g][:, t, :, :], in_=pp[:])
            # write k-projections (cols 64:128 of each (t, b) block) to HBM
            for t in range(NT):
                nc.sync.dma_start(
                    out=kph[
                        128 * t : 128 * (t + 1),
                        2 * D * g : 2 * D * (g + 1),
                    ],
                    in_=qk2[g][:, t, :, D:],
                )

        # ---- Phase B: attention per tile (gather all slices at once) ----
        outst = [
            opool.tile([P, NT, P], f32, name=f"outst_{g}", tag=f"outst_{g}")
            for g in range(NG)
        ]
        for t in range(NT):
            kg6 = gpool.tile([P, K, SPC, D], bf16, tag="kg6")
            # kg6[r', j, :] = kproj6[neigh[128t + r', j], :]
            # (one 128-row indirect DMA per neighbor slot; larger offset
            # batches mis-execute on this runtime)
            for j in range(K):
                nc.gpsimd.indirect_dma_start(
                    out=kg6[:, j, :, :].rearrange("p s e -> p (s e)"),
                    out_offset=None,
                    in_=kph[:],
                    in_offset=bass.IndirectOffsetOnAxis(
                        ap=gidx_sb[:, t, j : j + 1], axis=0
                    ),
                )
            for g in range(NG):
                kg = kg6[:, :, 2 * g : 2 * (g + 1), :]  # (P, K, 2, D)
                q2 = qk2[g][:, t, :, 0:D]  # (P, 2, D)
                q2b = q2.unsqueeze(1).to_broadcast([P, K, 2, D])

                prod = wpool.tile([P, K, 2, D], bf16, tag="prod")
                nc.vector.tensor_tensor(
                    out=prod[:], in0=kg[:], in1=q2b, op=OP.mult
                )
                scores = spool.tile([P, K, 2], f32, tag="scores")
                nc.vector.tensor_reduce(
                    out=scores[:], in_=prod[:], axis=AX.X, op=OP.add
                )
                negmax = spool.tile([P, 2], f32, tag="negmax")
                nc.vector.tensor_reduce(
                    out=negmax[:],
                    in_=scores[:].transpose([0, 2, 1]),
                    axis=AX.X,
                    op=OP.max,
                    negate=True,
                )
                expw = spool.tile([P, 2, K], f32, tag="expw")
                sumexp = spool.tile([P, 2], f32, tag="sumexp")
                for b in range(2):
                    nc.scalar.activation(
                        out=expw[:, b, :],
                        in_=scores[:, :, b],
                        func=AF.Exp,
                        bias=negmax[:, b : b + 1],
                        scale=1.0,
                        accum_out=sumexp[:, b : b + 1],
                    )
                rec = spool.tile([P, 2], f32, tag="rec")
                nc.vector.reciprocal(out=rec[:], in_=sumexp[:])

                # weighted kg, written transposed to (b, e, j) so the j-sum
                # is an innermost contiguous reduce
                wkg = wpool.tile([P, 2, D, K], bf16, tag="wkg")
                expb = expw[:].transpose([0, 2, 1]).unsqueeze(3).to_broadcast(
                    [P, K, 2, D]
                )
                nc.vector.tensor_tensor(
                    out=wkg[:].rearrange("p b e j -> p j b e"),
                    in0=kg[:],
                    in1=expb,
                    op=OP.mult,
                )
                ctxu = wpool.tile([P, 2, D], f32, tag="ctxu")
                nc.vector.tensor_reduce(
                    out=ctxu[:], in_=wkg[:], axis=AX.X, op=OP.add
                )
                # normalize by 1/sumexp and add q (residual onto projected q)
                ctxn = wpool.tile([P, 2, D], f32, tag="ctxn")
                for b in range(2):
                    nc.scalar.activation(
                        out=ctxn[:, b, :],
                        in_=ctxu[:, b, :],
                        func=AF.Copy,
                        scale=rec[:, b : b + 1],
                    )
                pre = wpool.tile([P, 2, D], bf16, tag="pre")
                nc.vector.tensor_tensor(
                    out=pre[:], in0=ctxn[:], in1=q2, op=OP.add
                )
                # transpose (r', (b,e)) -> ((b,e), r')
                pst = pspool.tile([P, P], bf16, tag="pst")
                nc.tensor.transpose(
                    out=pst[:],
                    in_=pre[:].rearrange("p b e -> p (b e)"),
                    identity=ident_bf[:],
                )
                preT = wpool.tile([P, P], bf16, tag="preT")
                nc.scalar.copy(out=preT[:], in_=pst[:])
                # final projection, both packed slices via block-diag wds2
                psf = pspool.tile([P, P], f32, tag="psf")
                nc.tensor.matmul(
                    psf[:], wds2[:], preT[:], start=True, stop=True
                )
                sigT = wpool.tile([P, P], f32, tag="sigT")
                nc.scalar.activation(
                    out=sigT[:], in_=psf[:], func=AF.Sigmoid
                )
                # transpose back to (r', (b, e))
                psb = pspool.tile([P, P], f32, tag="psb")
                nc.tensor.transpose(out=psb[:], in_=sigT[:], identity=ident_f32[:])
                nc.vector.tensor_copy(out=outst[g][:, t, :], in_=psb[:])
        for g in range(NG):
            nc.sync.dma_start(
                out=outs[g][:].rearrange("(t rp) e -> rp t e", t=NT),
                in_=outst[g][:],
            )

    return nc


def _prep_inputs(data, neigh_index):
    """Build per-core input maps."""
    dflat = np.ascontiguousarray(data.reshape(NBT, R, D))
    # gidx[t, r', j] = neigh[128t + r', j]
    gidx_all = np.ascontiguousarray(
        np.asarray(neigh_index).astype(np.int32).reshape(NT, 128, K)
    )
    return dflat, gidx_all


def kernel(data, neigh_index, wq, wk, wd_s):
    from concourse.bass_utils import run_bass_kernel_spmd

    data = np.asarray(data, dtype=np.float32)
    wq = np.asarray(wq, dtype=np.float32)
    wk = np.asarray(wk, dtype=np.float32)
    wd_s = np.asarray(wd_s, dtype=np.float32)

    dflat, gidx_all = _prep_inputs(data, neigh_index)
    wqk = np.concatenate([wq, wk], axis=1)  # (64, 128)

    nc = _build_bass()
    _hoist_multiwaits(nc)  # HW/walrus compat; CoreSim runs the unhoisted IR

    in_maps = []
    for c in range(NCORES):
        m = {"gidx": gidx_all, "wqk": wqk, "wds": wd_s}
        for g in range(NG):
            s0 = SPC * c + 2 * g
            # (128, 2048): two slices' transposed data stacked on partitions
            m[f"dat2T_g{g}"] = np.ascontiguousarray(
                np.concatenate([dflat[s0].T, dflat[s0 + 1].T], axis=0)
            )
        in_maps.append(m)

    res = run_bass_kernel_spmd(nc, in_maps, core_ids=list(range(NCORES)))
    global LAST_RESULTS
    LAST_RESULTS = res

    out = np.empty((NBT, R, D), dtype=np.float32)
    for c in range(NCORES):
        for g in range(NG):
            s0 = SPC * c + 2 * g
            og = res.results[c][f"out_g{g}"]  # (R, 128)
            out[s0] = og[:, :D]
            out[s0 + 1] = og[:, D:]
    return out.reshape(B, T, R, D)


def bench(data, neigh_index, wq, wk, wd_s, runs=5):
    """Build once, execute `runs` times on the 8 cores, return (out, times)."""
    import time

    import jax
    import jax.numpy as jnp
    from jax.sharding import Mesh, PartitionSpec
    from jax.experimental.shard_map import shard_map

    import concourse.bass2jax as bass2jax
    import concourse.mybir as mybir
    from concourse.bass2jax import _bass_exec_p, partition_id_tensor

    data = np.asarray(data, dtype=np.float32)
    dflat, gidx_all = _prep_inputs(data, neigh_index)
    wqk = np.concatenate(
        [np.asarray(wq, np.float32), np.asarray(wk, np.float32)], axis=1
    )
    nc = _build_bass()
    _hoist_multiwaits(nc)

    in_maps = []
    for c in range(NCORES):
        m = {"gidx": gidx_all, "wqk": wqk,
             "wds": np.asarray(wd_s, np.float32)}
        for g in range(NG):
            s0 = SPC * c + 2 * g
            m[f"dat2T_g{g}"] = np.ascontiguousarray(
                np.concatenate([dflat[s0].T, dflat[s0 + 1].T], axis=0)
            )
        in_maps.append(m)

    in_names, out_names, out_avals, zero_outs = [], [], [], []
    partition_name = (
        nc.partition_id_tensor.name if nc.partition_id_tensor else None
    )
    for alloc in nc.m.functions[0].allocations:
        if not isinstance(alloc, mybir.MemoryLocationSet):
            continue
        name = alloc.memorylocations[0].name
        if alloc.kind == "ExternalInput":
            if name != partition_name:
                in_names.append(name)
        elif alloc.kind == "ExternalOutput":
            out_names.append(name)
            shape = tuple(alloc.tensor_shape)
            dtype = mybir.dt.np(alloc.dtype)
            out_avals.append(jax.core.ShapedArray(shape, dtype))
            zero_outs.append(np.zeros(shape, dtype))
    n_params = len(in_names)
    n_outs = len(out_avals)
    all_in_names = in_names + out_names + (
        [partition_name] if partition_name else []
    )

    def _body(*args):
        operands = list(args)
        if partition_name is not None:
            operands.append(partition_id_tensor())
        return tuple(
            _bass_exec_p.bind(
                *operands,
                out_avals=tuple(out_avals),
                in_names=tuple(all_in_names),
                out_names=tuple(out_names),
                lowering_input_output_aliases=(),
                sim_require_finite=True,
                sim_require_nnan=True,
                nc=nc,
            )
        )

    devices = jax.devices()[:NCORES]
    mesh = Mesh(np.asarray(devices), ("core",))
    sharded = jax.jit(
        shard_map(
            _body,
            mesh=mesh,
            in_specs=(PartitionSpec("core"),) * (n_params + n_outs),
            out_specs=(PartitionSpec("core"),) * n_outs,
            check_rep=False,
        ),
        donate_argnums=tuple(range(n_params, n_params + n_outs)),
        keep_unused=True,
    )
    concat_in = [
        np.concatenate([np.asarray(in_maps[c][nm]) for c in range(NCORES)], 0)
        for nm in in_names
    ]
    times = []
    out_arrs = None
    for r in range(runs):
        concat_zeros = [
            np.zeros((NCORES * z.shape[0], *z.shape[1:]), z.dtype)
            for z in zero_outs
        ]
        zdev = jax.device_put(concat_zeros)
        indev = jax.device_put(concat_in)
        jax.block_until_ready(zdev)
        jax.block_until_ready(indev)
        t0 = time.perf_counter()
        out_arrs = sharded(*indev, *zdev)
        jax.block_until_ready(out_arrs)
        times.append(time.perf_counter() - t0)

    out = np.empty((NBT, R, D), dtype=np.float32)
    for c in range(NCORES):
        for g in range(NG):
            s0 = SPC * c + 2 * g
            i = out_names.index(f"out_g{g}")
            og = np.asarray(out_arrs[i]).reshape(NCORES, R, P)[c]
            out[s0] = og[:, :D]
            out[s0 + 1] = og[:, D:]
    return out.reshape(B, T, R, D), times


if __name__ == "__main__":
    rng = np.random.default_rng(0)
    data = rng.standard_normal((B, T, R, D), dtype=np.float32)
    neigh = rng.integers(0, R, size=(R, K)).astype(np.int32)
    wq = (0.01 + 0.005 * rng.standard_normal((D, D))).astype(np.float32)
    wk = (0.01 + 0.005 * rng.standard_normal((D, D))).astype(np.float32)
    wd_s = (0.01 + 0.005 * rng.standard_normal((D, D))).astype(np.float32)
    out = kernel(data=data, neigh_index=neigh, wq=wq, wk=wk, wd_s=wd_s)
    print(out.shape, out.dtype)



# revision 2
# speedup vs baseline: 1.0946x; 1.0946x over previous
"""Trainium2 Bass kernel for nn_Attention_84851373900515 (gnn message passing).

Reference computation (per (b, t) slice, R=2048 regions, D=64, K=16 neighbors):
    q = data @ wq                       # (R, D)
    k = data[neigh] @ wk = (data @ wk)[neigh]   # project-then-gather
    scores[r, j] = q[r] . k[neigh[r, j]]
    attn = softmax_j(scores)
    ctx[r] = sum_j attn[r, j] * k[neigh[r, j]]
    out = sigmoid((q + ctx) @ wd_s)

Sharding: data-parallel over the 48 (b, t) slices, 6 per core, processed as
3 groups of 2 slices packed side by side ("b2" packing) so gathered rows are
256 bytes (dma_gather minimum).

Pipeline per core:
  A. PE computes q/k projections from host-pretransposed data (bf16),
     k-projections are written back to a per-core HBM scratch.
  B. Per (group, region-tile): dma_gather pulls the 2048 neighbor rows
     (16 per region) from HBM into SBUF; DVE computes scores (mult+reduce),
     ACT does exp (softmax), DVE/ACT compute the attention-weighted context,
     PE transposes and applies the final wd_s projection, ACT applies
     sigmoid, PE transposes back, and the result is DMA'd out fp32.
"""

import sys

sys.path.insert(0, "/opt/trn_rl_repo")

import numpy as np

LAST_RESULTS = None  # BassKernelResults of the most recent kernel() call

B, T, R, D, K = 4, 12, 2048, 64, 16
NBT = B * T          # 48 (b, t) slices
NCORES = 8
SPC = NBT // NCORES  # 6 slices per core
NG = SPC // 2        # 3 groups of 2 slices
NT = R // 128        # 16 region tiles
P = 128


def _patch_tile_compat():
    """The walrus bundled with the installed neuronxcc (which the axon
    bass2jax path compiles through) cannot encode (a) the raw-ISA
    EVENT_SEMAPHORE_RANGE_CLEAR instruction and (b) control instructions
    carrying more than one semaphore wait. Patch Tile's kernel tail:
    skip the semaphore/DMA hardware reset (each compiled NEFF here runs
    exactly once) and split the tail drain's accumulated waits into
    single-wait EventSemaphore instructions."""
    import concourse.bass as bass
    import concourse.mybir as mybir
    import concourse.tile as tile
    from concourse.vector_clock import ScopedClock

    if getattr(tile.TileContext, "_ant_compat_patched", False):
        return

    def clear_and_free(self, sems):
        if not sems:
            return
        sem_nums = [s.num if hasattr(s, "num") else s for s in sems]
        self._state.prepend_free_semaphores(sem_nums)
        for poison_set in self._tile_sem_poison_stack:
            poison_set.update(sem_nums)

    bass.Bass.clear_and_free_semaphores = clear_and_free

    def drain_and_barrier(self, tick_clock, wait_clock):
        nc = self.nc
        drain_inst = nc.sync.drain()
        wait_clock.add_sem_waits(
            drain_inst.ins, ScopedClock({None: tick_clock.global_clock})
        )
        mi = drain_inst.ins
        si = mi.sync_info
        if si is not None and len(si.on_wait) > 1:
            waits = list(si.on_wait)
            mi.sync_info = mybir.SyncInfo(
                on_wait=[], on_update=list(si.on_update)
            )
            for w in waits:
                ev = mybir.InstEventSemaphore(
                    name=nc.get_next_instruction_name(),
                    engine=mybir.EngineType.SP,
                    ins=[],
                    outs=[],
                    sync_info=mybir.SyncInfo(on_wait=[w], on_update=[]),
                )
                self._add_instruction(ev)
        nc.all_engine_barrier()
        assert self.sems is not None
        popped = nc._tile_sem_poison_stack.pop()
        assert popped is self._sem_poison
        nc.clear_and_free_semaphores(list(self.sems.allocated().values()))
        nc.all_engine_barrier()

    tile.TileContext._drain_and_barrier = drain_and_barrier
    tile.TileContext._ant_compat_patched = True


def _hoist_multiwaits(nc):
    """Split semaphore waits that exceed what the installed walrus can
    encode per instruction into standalone single-wait EventSemaphore
    instructions on the same engine, inserted immediately before."""
    import concourse.mybir as mybir

    for f in nc.m.functions:
        for blk in f.blocks:
            out = []
            changed = False
            for inst in blk.instructions:
                si = inst.sync_info
                limit = 1
                if si is not None and len(si.on_wait) > limit:
                    waits = list(si.on_wait)
                    keep, hoist = waits[:limit], waits[limit:]
                    for w in hoist:
                        ev = mybir.InstEventSemaphore(
                            name=nc.get_next_instruction_name(),
                            engine=inst.engine,
                            ins=[],
                            outs=[],
                            sync_info=mybir.SyncInfo(on_wait=[w], on_update=[]),
                        )
                        out.append(ev)
                    inst.sync_info = mybir.SyncInfo(
                        on_wait=keep, on_update=list(si.on_update)
                    )
                    changed = True
                out.append(inst)
            if changed:
                blk.instructions = out


def _build_bass():
    from contextlib import ExitStack

    import concourse.bass as bass
    import concourse.mybir as mybir
    import concourse.tile as tile
    from concourse.masks import make_identity

    _patch_tile_compat()

    f32 = mybir.dt.float32
    bf16 = mybir.dt.bfloat16
    i32 = mybir.dt.int32
    AF = mybir.ActivationFunctionType
    OP = mybir.AluOpType
    AX = mybir.AxisListType

    nc = bass.Bass()

    # Inputs (per core)
    dat2T = [
        nc.declare_dram_parameter(f"dat2T_g{g}", [P, R], f32, isOutput=False)
        for g in range(NG)
    ]
    gidx = nc.declare_dram_parameter("gidx", [NT, P, K], i32, isOutput=False)
    wqk = nc.declare_dram_parameter("wqk", [D, 2 * D], f32, isOutput=False)
    wds = nc.declare_dram_parameter("wds", [D, D], f32, isOutput=False)
    outs = [
        nc.declare_dram_parameter(f"out_g{g}", [R, P], f32, isOutput=True)
        for g in range(NG)
    ]
    # HBM scratch holding the bf16 k-projections, gather source. Row r holds
    # all six slices' k vectors: (r, (s6, e)) = 768 B, so one 128-offset
    # indirect DMA gathers a neighbor slot for every slice at once.
    kph = nc.dram_tensor("kproj6", [R, SPC * D], bf16)

    with ExitStack() as ctx:
        tc = ctx.enter_context(tile.TileContext(nc))
        cpool = ctx.enter_context(tc.tile_pool(name="consts", bufs=1))
        apool = ctx.enter_context(tc.tile_pool(name="phaseA", bufs=2))
        qkpool = ctx.enter_context(tc.tile_pool(name="qk", bufs=1))
        gpool = ctx.enter_context(tc.tile_pool(name="gather", bufs=3))
        wpool = ctx.enter_context(tc.tile_pool(name="work", bufs=3))
        spool = ctx.enter_context(tc.tile_pool(name="small", bufs=4))
        opool = ctx.enter_context(tc.tile_pool(name="outst", bufs=1))
        pspool = ctx.enter_context(tc.tile_pool(name="ps", bufs=2, space="PSUM"))

        # ---- constants ----
        ident_bf = cpool.tile([P, P], bf16)
        make_identity(nc, ident_bf[:])
        ident_f32 = cpool.tile([P, P], f32)
        make_identity(nc, ident_f32[:])

        # Block-diagonal weights so both packed slices project in one matmul
        # (contract over all 128 partitions = (b2, e)).
        wqk_f = cpool.tile([D, 2 * D], f32)
        nc.sync.dma_start(out=wqk_f[:], in_=wqk[:])
        w2 = cpool.tile([P, 4 * D], bf16)
        nc.vector.memset(w2[:], 0.0)
        nc.vector.tensor_copy(out=w2[0:D, 0 : 2 * D], in_=wqk_f[:])
        nc.vector.tensor_copy(out=w2[D:P, 2 * D : 4 * D], in_=wqk_f[:])
        wds_f = cpool.tile([D, D], f32)
        nc.sync.dma_start(out=wds_f[:], in_=wds[:])
        wds2 = cpool.tile([P, P], bf16)
        nc.vector.memset(wds2[:], 0.0)
        nc.vector.tensor_copy(out=wds2[0:D, 0:D], in_=wds_f[:])
        nc.vector.tensor_copy(out=wds2[D:P, D:P], in_=wds_f[:])

        gidx_sb = cpool.tile([P, NT, K], i32)
        nc.sync.dma_start(
            out=gidx_sb[:], in_=gidx[:].rearrange("t rp j -> rp t j")
        )

        # qk2[r', t, b, (qk, e)] bf16 for the current group
        qk2 = [
            qkpool.tile([P, NT, 2, P], bf16, name=f"qk2_{g}", tag=f"qk2_{g}")
            for g in range(NG)
        ]

        # ---- Phase A: projections for all groups ----
        for g in range(NG):
            d2t = apool.tile([P, R], bf16, tag="d2t")
            # SWDGE dma casts fp32 -> bf16 on the way in
            nc.gpsimd.dma_start(out=d2t[:], in_=dat2T[g][:])
            for t in range(NT):
                pp = pspool.tile([P, 2, P], f32, tag="proj")
                # out[r', (b, qk, e)] = sum_(b', e') d2t[(b', e'), r'] *
                #                       w2[(b', e'), (b, qk, e)]
                nc.tensor.matmul(
                    pp[:],
                    d2t[:, 128 * t : 128 * (t + 1)],
                    w2[:],
                    start=True,
                    stop=True,
                )
                if t % 2 == 0:
                    nc.vector.tensor_copy(out=qk2[g][:, t, :, :], in_=pp[:])
                else:
                    nc.scalar.copy(out=qk2[g][:, t, :, :], in_=pp[:])
            # write k-projections (cols 64:128 of each (t, b) block) to HBM
            for t in range(NT):
                nc.sync.dma_start(
                    out=kph[
                        128 * t : 128 * (t + 1),
                        2 * D * g : 2 * D * (g + 1),
                    ],
                    in_=qk2[g][:, t, :, D:],
                )

        # ---- Phase B: attention per tile (gather all slices at once) ----
        outst = [
            opool.tile([P, NT, P], f32, name=f"outst_{g}", tag=f"outst_{g}")
            for g in range(NG)
        ]
        for t in range(NT):
            kg6 = gpool.tile([P, K, SPC, D], bf16, tag="kg6")
            # kg6[r', j, :] = kproj6[neigh[128t + r', j], :]
            # (one 128-row indirect DMA per neighbor slot; larger offset
            # batches mis-execute on this runtime)
            for j in range(K):
                nc.gpsimd.indirect_dma_start(
                    out=kg6[:, j, :, :].rearrange("p s e -> p (s e)"),
                    out_offset=None,
                    in_=kph[:],
                    in_offset=bass.IndirectOffsetOnAxis(
                        ap=gidx_sb[:, t, j : j + 1], axis=0
                    ),
                )
            for g in range(NG):
                kg = kg6[:, :, 2 * g : 2 * (g + 1), :]  # (P, K, 2, D)
                q2 = qk2[g][:, t, :, 0:D]  # (P, 2, D)
                q2b = q2.unsqueeze(1).to_broadcast([P, K, 2, D])

                prod = wpool.tile([P, K, 2, D], bf16, tag="prod")
                nc.vector.tensor_tensor(
                    out=prod[:], in0=kg[:], in1=q2b, op=OP.mult
                )
                scores = spool.tile([P, K, 2], f32, tag="scores")
                nc.vector.tensor_reduce(
                    out=scores[:], in_=prod[:], axis=AX.X, op=OP.add
                )
                negmax = spool.tile([P, 2], f32, tag="negmax")
                nc.vector.tensor_reduce(
                    out=negmax[:],
                    in_=scores[:].transpose([0, 2, 1]),
                    axis=AX.X,
                    op=OP.max,
                    negate=True,
                )
                expw = spool.tile([P, 2, K], f32, tag="expw")
                sumexp = spool.tile([P, 2], f32, tag="sumexp")
                for b in range(2):
                    nc.scalar.activation(
                        out=expw[:, b, :],
                        in_=scores[:, :, b],
                        func=AF.Exp,
                        bias=negmax[:, b : b + 1],
                        scale=1.0,
                        accum_out=sumexp[:, b : b + 1],
                    )
                rec = spool.tile([P, 2], f32, tag="rec")
                nc.vector.reciprocal(out=rec[:], in_=sumexp[:])

                # weighted kg, written transposed to (b, e, j) so the j-sum
                # is an innermost contiguous reduce
                wkg = wpool.tile([P, 2, D, K], bf16, tag="wkg")
                expb = expw[:].transpose([0, 2, 1]).unsqueeze(3).to_broadcast(
                    [P, K, 2, D]
                )
                nc.vector.tensor_tensor(
                    out=wkg[:].rearrange("p b e j -> p j b e"),
                    in0=kg[:],
                    in1=expb,
                    op=OP.mult,
                )
                ctxu = wpool.tile([P, 2, D], f32, tag="ctxu")
                nc.vector.tensor_reduce(
                    out=ctxu[:], in_=wkg[:], axis=AX.X, op=OP.add
                )
                # normalize by 1/sumexp and add q (residual onto projected q)
                ctxn = wpool.tile([P, 2, D], f32, tag="ctxn")
                for b in range(2):
                    nc.scalar.activation(
                        out=ctxn[:, b, :],
                        in_=ctxu[:, b, :],
                        func=AF.Copy,
                        scale=rec[:, b : b + 1],
                    )
                pre = wpool.tile([P, 2, D], bf16, tag="pre")
                nc.vector.tensor_tensor(
                    out=pre[:], in0=ctxn[:], in1=q2, op=OP.add
                )
                # transpose (r', (b,e)) -> ((b,e), r')
                pst = pspool.tile([P, P], bf16, tag="pst")
                nc.tensor.transpose(
                    out=pst[:],
                    in_=pre[:].rearrange("p b e -> p (b e)"),
                    identity=ident_bf[:],
                )
                preT = wpool.tile([P, P], bf16, tag="preT")
                nc.scalar.copy(out=preT[:], in_=pst[:])
                # final projection, both packed slices via block-diag wds2
                psf = pspool.tile([P, P], f32, tag="psf")
                nc.tensor.matmul(
                    psf[:], wds2[:], preT[:], start=True, stop=True
                )
                sigT = wpool.tile([P, P], f32, tag="sigT")
                nc.scalar.activation(
                    out=sigT[:], in_=psf[:], func=AF.Sigmoid
                )
                # transpose back to (r', (b, e))
                psb = pspool.tile([P, P], f32, tag="psb")
                nc.tensor.transpose(out=psb[:], in_=sigT[:], identity=ident_f32[:])
                nc.vector.tensor_copy(out=outst[g][:, t, :], in_=psb[:])
        for g in range(NG):
            nc.sync.dma_start(
                out=outs[g][:].rearrange("(t rp) e -> rp t e", t=NT),
                in_=outst[g][:],
            )

    return nc


def _prep_inputs(data, neigh_index):
    """Build per-core input maps."""
    dflat = np.ascontiguousarray(data.reshape(NBT, R, D))
    # gidx[t, r', j] = neigh[128t + r', j]
    gidx_all = np.ascontiguousarray(
        np.asarray(neigh_index).astype(np.int32).reshape(NT, 128, K)
    )
    return dflat, gidx_all


def kernel(data, neigh_index, wq, wk, wd_s):
    from concourse.bass_utils import run_bass_kernel_spmd

    data = np.asarray(data, dtype=np.float32)
    wq = np.asarray(wq, dtype=np.float32)
    wk = np.asarray(wk, dtype=np.float32)
    wd_s = np.asarray(wd_s, dtype=np.float32)

    dflat, gidx_all = _prep_inputs(data, neigh_index)
    wqk = np.concatenate([wq, wk], axis=1)  # (64, 128)

    nc = _build_bass()
    _hoist_multiwaits(nc)  # HW/walrus compat; CoreSim runs the unhoisted IR

    in_maps = []
    for c in range(NCORES):
        m = {"gidx": gidx_all, "wqk": wqk, "wds": wd_s}
        for g in range(NG):
            s0 = SPC * c + 2 * g
            # (128, 2048): two slices' transposed data stacked on partitions
            m[f"dat2T_g{g}"] = np.ascontiguousarray(
                np.concatenate([dflat[s0].T, dflat[s0 + 1].T], axis=0)
            )
        in_maps.append(m)

    res = run_bass_kernel_spmd(nc, in_maps, core_ids=list(range(NCORES)))
    global LAST_RESULTS
    LAST_RESULTS = res

    out = np.empty((NBT, R, D), dtype=np.float32)
    for c in range(NCORES):
        for g in range(NG):
            s0 = SPC * c + 2 * g
            og = res.results[c][f"out_g{g}"]  # (R, 128)
            out[s0] = og[:, :D]
            out[s0 + 1] = og[:, D:]
    return out.reshape(B, T, R, D)


def bench(data, neigh_index, wq, wk, wd_s, runs=5):
    """Build once, execute `runs` times on the 8 cores, return (out, times)."""
    import time

    import jax
    import jax.numpy as jnp
    from jax.sharding import Mesh, PartitionSpec
    from jax.experimental.shard_map import shard_map

    import concourse.bass2jax as bass2jax
    import concourse.mybir as mybir
    from concourse.bass2jax import _bass_exec_p, partition_id_tensor

    data = np.asarray(data, dtype=np.float32)
    dflat, gidx_all = _prep_inputs(data, neigh_index)
    wqk = np.concatenate(
        [np.asarray(wq, np.float32), np.asarray(wk, np.float32)], axis=1
    )
    nc = _build_bass()
    _hoist_multiwaits(nc)

    in_maps = []
    for c in range(NCORES):
        m = {"gidx": gidx_all, "wqk": wqk,
             "wds": np.asarray(wd_s, np.float32)}
        for g in range(NG):
            s0 = SPC * c + 2 * g
            m[f"dat2T_g{g}"] = np.ascontiguousarray(
                np.concatenate([dflat[s0].T, dflat[s0 + 1].T], axis=0)
            )
        in_maps.append(m)

    in_names, out_names, out_avals, zero_outs = [], [], [], []
    partition_name = (
        nc.partition_id_tensor.name if nc.partition_id_tensor else None
    )
    for alloc in nc.m.functions[0].allocations:
        if not isinstance(alloc, mybir.MemoryLocationSet):
            continue
        name = alloc.memorylocations[0].name
        if alloc.kind == "ExternalInput":
            if name != partition_name:
                in_names.append(name)
        elif alloc.kind == "ExternalOutput":
            out_names.append(name)
            shape = tuple(alloc.tensor_shape)
            dtype = mybir.dt.np(alloc.dtype)
            out_avals.append(jax.core.ShapedArray(shape, dtype))
            zero_outs.append(np.zeros(shape, dtype))
    n_params = len(in_names)
    n_outs = len(out_avals)
    all_in_names = in_names + out_names + (
        [partition_name] if partition_name else []
    )

    def _body(*args):
        operands = list(args)
        if partition_name is not None:
            operands.append(partition_id_tensor())
        return tuple(
            _bass_exec_p.bind(
                *operands,
                out_avals=tuple(out_avals),
                in_names=tuple(all_in_names),
                out_names=tuple(out_names),
                lowering_input_output_aliases=(),
                sim_require_finite=True,
                sim_require_nnan=True,
                nc=nc,
            )
        )

    devices = jax.devices()[:NCORES]
    mesh = Mesh(np.asarray(devices), ("core",))
    sharded = jax.jit(
        shard_map(
            _body,
            mesh=mesh,
            in_specs=(PartitionSpec("core"),) * (n_params + n_outs),
            out_specs=(PartitionSpec("core"),) * n_outs,
            check_rep=False,
        ),
        donate_argnums=tuple(range(n_params, n_params + n_outs)),
        keep_unused=True,
    )
    from jax.sharding import NamedSharding

    shard = NamedSharding(mesh, PartitionSpec("core"))
    concat_in = [
        np.concatenate([np.asarray(in_maps[c][nm]) for c in range(NCORES)], 0)
        for nm in in_names
    ]
    times = []
    out_arrs = None
    for r in range(runs):
        concat_zeros = [
            np.zeros((NCORES * z.shape[0], *z.shape[1:]), z.dtype)
            for z in zero_outs
        ]
        zdev = [jax.device_put(z, shard) for z in concat_zeros]
        indev = [jax.device_put(a, shard) for a in concat_in]
        jax.block_until_ready(zdev)
        jax.block_until_ready(indev)
        t0 = time.perf_counter()
        out_arrs = sharded(*indev, *zdev)
        jax.block_until_ready(out_arrs)
        times.append(time.perf_counter() - t0)

    out = np.empty((NBT, R, D), dtype=np.float32)
    for c in range(NCORES):
        for g in range(NG):
            s0 = SPC * c + 2 * g
            i = out_names.index(f"out_g{g}")
            og = np.asarray(out_arrs[i]).reshape(NCORES, R, P)[c]
            out[s0] = og[:, :D]
            out[s0 + 1] = og[:, D:]
    return out.reshape(B, T, R, D), times


if __name__ == "__main__":
    rng = np.random.default_rng(0)
    data = rng.standard_normal((B, T, R, D), dtype=np.float32)
    neigh = rng.integers(0, R, size=(R, K)).astype(np.int32)
    wq = (0.01 + 0.005 * rng.standard_normal((D, D))).astype(np.float32)
    wk = (0.01 + 0.005 * rng.standard_normal((D, D))).astype(np.float32)
    wd_s = (0.01 + 0.005 * rng.standard_normal((D, D))).astype(np.float32)
    out = kernel(data=data, neigh_index=neigh, wq=wq, wk=wk, wd_s=wd_s)
    print(out.shape, out.dtype)

